# revision 17
# baseline (speedup 1.0000x reference)
"""Trainium2 Bass kernel for nn_BidirectionalBoxPool.

Contract: kernel(x, boxes) takes FULL inputs (x: (8,128,128,128) f32,
boxes: (8,64,4) f32) and returns (feats, widths) matching the reference:
feats (8, 64, 2, 128, 8, MW) f32, widths (8, 64, 2) f32, with MW the
data-dependent max pooled width.

Strategy: data-parallel over the batch axis N — core n handles image n.

Math per image: grid_sample with a per-box separable bilinear grid, so
  feats[k,d,c,i,j] = sum_h sum_w img[c,h,w] * wy_k[h,i] * wx_k[w,j']
with the dir-1 grid being an exact (i,j)-flip of dir-0 within each box's
valid width. Host code (numpy) replicates the reference's fp32 grid math
exactly and bakes it into two small weight tensors per image:
  WY  [h=128, K*8]     y-interp weights (f32, fed to fp32r matmuls)
  WXF [w=128, K*2*MW]  x-interp weights, dir0 block + flipped dir1 block (f16)
Device program (SPMD identical across cores; all box data flows through
the weight tensors):
  stage 1: per channel c: PSUM[w,(k,i)] = img_c[h,w]^T @ WY   (fp32r)
           -> SBUF S[w, (k,i)*C+c] in fp16
  stage 2: per (k,i): PSUM2[c, cols] = S_ki[w,c]^T @ WXF_k    (fp16)
           two matmuls (dir0/dir1) into bank-aligned PSUM halves
  out: per box, copy PSUM2 -> SBUF [c,(d,i,j)] and DMA to DRAM feats[k].
Tall boxes (bh > bw, ~7%, width<=16) have their grid transposed relative
to the wide layout; they are zeroed on device (zero weights) and patched
in exactly on host, as is the `widths` output.
"""

from contextlib import ExitStack

import numpy as np

import concourse.bass as bass
import concourse.tile as tile
from concourse import bacc, bass_utils, mybir

F32 = mybir.dt.float32
F32R = mybir.dt.float32r
F16 = mybir.dt.float16

PH = 8
N_CORES = 8
NPF32 = np.float32


# ----------------------------------------------------------------------------
# host-side weight construction (replicates reference fp32 grid math)
# ----------------------------------------------------------------------------

def _box_meta(boxes):
    b = boxes.astype(NPF32)
    xmin, ymin, xmax, ymax = b[:, 0], b[:, 1], b[:, 2], b[:, 3]
    valid = ~((xmin == 0) & (ymin == 0) & (xmax == 0) & (ymax == 0))
    one = NPF32(1.0)
    bw = np.where(valid, (xmax - xmin).astype(NPF32), one).astype(NPF32)
    bh = np.where(valid, (ymax - ymin).astype(NPF32), one).astype(NPF32)
    wide = bw > bh
    ratio = np.where(wide, (bw / bh).astype(NPF32),
                     (bh / bw).astype(NPF32)).astype(NPF32)
    width = np.ceil((ratio * NPF32(PH)).astype(NPF32)).astype(np.int32)
    width = np.where(valid, width, 0)
    wf = np.maximum(width, 2).astype(NPF32)
    return valid, wide, width, wf, bw, bh, xmin.astype(NPF32), ymin.astype(NPF32)


def _max_width(boxes_all):
    b = np.asarray(boxes_all, dtype=np.float64)
    valid = ~np.all(b == 0, axis=-1)
    bw = np.where(valid, b[..., 2] - b[..., 0], 1.0)
    bh = np.where(valid, b[..., 3] - b[..., 1], 1.0)
    ratio = np.where(bw > bh, bw / bh, bh / bw)
    ratio = np.where(valid, ratio, 0.0)
    return int(np.ceil(ratio.max() * PH))


def _grid_wide(xmin, ymin, bw, bh, wf, W, H, ii, jj):
    gx = ((xmin + (jj * bw / (wf - NPF32(1.0))).astype(NPF32)).astype(NPF32)
          - NPF32(W / 2)) / NPF32(W / 2)
    gy = ((ymin + (ii * bh / NPF32(PH - 1.0)).astype(NPF32)).astype(NPF32)
          - NPF32(H / 2)) / NPF32(H / 2)
    return gx.astype(NPF32), gy.astype(NPF32)


def _grid_tall(xmin, ymin, bw, bh, wf, W, H, ii, jj):
    gx = ((xmin + (ii * bw / NPF32(PH - 1.0)).astype(NPF32)).astype(NPF32)
          - NPF32(W / 2)) / NPF32(W / 2)
    gy = ((ymin + ((wf - jj) * bh / (wf - NPF32(1.0))).astype(NPF32)).astype(NPF32)
          - NPF32(H / 2)) / NPF32(H / 2)
    return gx.astype(NPF32), gy.astype(NPF32)


def _taps(g, n):
    g = g.astype(NPF32)
    pos = ((g + NPF32(1.0)) * NPF32(n) - NPF32(1.0)) * NPF32(0.5)
    pos64 = pos.astype(np.float64)
    i0 = np.floor(pos64).astype(np.int64)
    f = pos64 - i0
    w0 = np.where((i0 >= 0) & (i0 <= n - 1), 1.0 - f, 0.0)
    w1 = np.where((i0 + 1 >= 0) & (i0 + 1 <= n - 1), f, 0.0)
    return i0, w0, w1


def _build_image_weights(boxes, H, W, MW):
    K = boxes.shape[0]
    valid, wide, width, wf, bw, bh, xmin, ymin = _box_meta(boxes)
    WY = np.zeros((H, K * PH), np.float64)
    WXF = np.zeros((W, K * 2 * MW), np.float64)
    tall_idx = []
    ii = np.arange(PH, dtype=NPF32)
    for k in range(K):
        if not valid[k]:
            continue
        if not wide[k]:
            tall_idx.append(k)
            continue
        wk = int(width[k])
        jj = np.arange(wk, dtype=NPF32)
        gx, gy = _grid_wide(xmin[k], ymin[k], bw[k], bh[k], wf[k], W, H, ii, jj)
        y0, wy0, wy1 = _taps(gy, H)
        for i in range(PH):
            col = k * PH + i
            if wy0[i] != 0.0:
                WY[y0[i], col] += wy0[i]
            if wy1[i] != 0.0:
                WY[y0[i] + 1, col] += wy1[i]
        x0, wx0, wx1 = _taps(gx, W)
        base = k * 2 * MW
        for j in range(min(wk, MW)):
            if wx0[j] != 0.0:
                WXF[x0[j], base + j] += wx0[j]
            if wx1[j] != 0.0:
                WXF[x0[j] + 1, base + j] += wx1[j]
            jr = wk - 1 - j
            if wx0[jr] != 0.0:
                WXF[x0[jr], base + MW + j] += wx0[jr]
            if wx1[jr] != 0.0:
                WXF[x0[jr] + 1, base + MW + j] += wx1[jr]
    return WY.astype(NPF32), WXF.astype(NPF32), width, tall_idx


def _tall_feats(img, boxes, k, H, W, MW):
    valid, wide, width, wf, bw, bh, xmin, ymin = _box_meta(boxes)
    C = img.shape[0]
    wk = int(width[k])
    out = np.zeros((2, C, PH, MW), NPF32)
    ii = np.arange(PH, dtype=NPF32)[:, None]
    jj = np.arange(wk, dtype=NPF32)[None, :]
    gx, gy = _grid_tall(xmin[k], ymin[k], bw[k], bh[k], wf[k], W, H, ii, jj)
    gx = np.broadcast_to(gx, (PH, wk))
    gy = np.broadcast_to(gy, (PH, wk))
    x0, wx0, wx1 = _taps(gx, W)
    y0, wy0, wy1 = _taps(gy, H)
    imgf = img.astype(np.float64)

    def gat(yc, xc, m):
        yi = np.clip(yc, 0, H - 1)
        xi = np.clip(xc, 0, W - 1)
        return imgf[:, yi, xi] * m

    s = (gat(y0, x0, wy0 * wx0) + gat(y0, x0 + 1, wy0 * wx1)
         + gat(y0 + 1, x0, wy1 * wx0) + gat(y0 + 1, x0 + 1, wy1 * wx1))
    wcl = min(wk, MW)
    out[0, :, :, :wcl] = s[:, :, :wcl].astype(NPF32)
    out[1, :, :, :wcl] = s[:, ::-1, ::-1][:, :, :wcl].astype(NPF32)
    return out


# ----------------------------------------------------------------------------
# device program
# ----------------------------------------------------------------------------

S1_DTYPE = F16  # stage-1 matmul dtype: F16 (fast) or F32R (higher precision)
OUT_F16 = True  # device writes fp16 feats; host upcasts


def _build_program_v4(C, K, MW, KG=None, P=128):
    """v4: stage-2 swapped — stationary = WXF_k (contiguous, 1 LDW/box),
    moving = S block (c-major sg, runs-of-8 strided AP, full rate).
    Output goes to a DRAM scratch [k, jd, (h, c', i)] (fp16, 2KB/partition
    DMA chunks); the final (k,d,c,i,j) permutation happens on host."""
    KG = K  # single group
    NW = 2 * MW
    NWP = 128  # wxf padded to 128 cols/box so DMA tiles have 128 partitions
    assert NW <= NWP
    G = K // KG
    assert K % KG == 0 and C % 2 == 0
    ncols = KG * PH
    CH = C // 2  # c-half per N=CH*PH matmul

    nc = bacc.Bacc("TRN2", target_bir_lowering=False, debug=False,
                   enable_asserts=True, num_devices=1)

    img = nc.dram_tensor("img", [P, C * P], F16, kind="ExternalInput").ap()
    wy = nc.dram_tensor("wy", [P, K * PH], F16, kind="ExternalInput").ap()
    wxf = nc.dram_tensor("wxf", [P, K * NWP], F16, kind="ExternalInput").ap()
    feats = nc.dram_tensor("feats", [K, NWP * C * PH], F16,
                           kind="ExternalOutput").ap()

    with tile.TileContext(nc) as tc, ExitStack() as ctx:
        const_pool = ctx.enter_context(tc.tile_pool(name="const", bufs=1))
        s_pool = ctx.enter_context(tc.tile_pool(name="sg", bufs=1))
        st_pool = ctx.enter_context(tc.tile_pool(name="st", bufs=4))
        ps1_pool = ctx.enter_context(tc.tile_pool(name="ps1", bufs=2, space="PSUM"))
        ps2_pool = ctx.enter_context(tc.tile_pool(name="ps2", bufs=2, space="PSUM"))

        img_t = const_pool.tile([P, C * P], F16)
        wy_t = const_pool.tile([P, K * PH], F16)
        wxf_t = const_pool.tile([P, K * NWP], F16)
        nc.sync.dma_start(wy_t[:], wy)
        nc.sync.dma_start(wxf_t[:], wxf)
        CCH = min(16, C)  # img DMA chunk (channels) so stage 1 starts early
        for cc in range(0, C, CCH):
            nc.sync.dma_start(img_t[:, cc * P:(cc + CCH) * P],
                              img[:, cc * P:(cc + CCH) * P])

        assert G == 1
        ncols = K * PH
        # stage 1: c-major S (free index = c*ncols + ki); two channels
        # share one two-bank ps1 tile so casts move 1024 columns at a time
        sg = s_pool.tile([P, C * ncols], F16, tag="sg")
        for c2 in range(C // 2):
            ps1 = ps1_pool.tile([P, 2 * ncols], F32, tag="ps1")
            for h in range(2):
                c = 2 * c2 + h
                nc.tensor.matmul(
                    ps1[:, h * ncols:(h + 1) * ncols],
                    img_t[:, c * P:(c + 1) * P],
                    wy_t[:],
                )
            if c2 % 2 == 0:
                nc.vector.tensor_copy(
                    sg[:, 2 * c2 * ncols:(2 * c2 + 2) * ncols], ps1[:])
            else:
                nc.scalar.copy(
                    sg[:, 2 * c2 * ncols:(2 * c2 + 2) * ncols], ps1[:])

        # stage 2 (swapped): per box, one LDW + 2 matmuls N=CH*PH
        for k in range(K):
            st = st_pool.tile([NWP, C * PH], F16, tag="st")
            ps2 = ps2_pool.tile([NWP, C * PH], F32, tag="ps2")
            for h in range(2):
                mov = bass.AP(sg.tensor,
                              sg[:].offset + h * CH * ncols + k * PH,
                              [[sg[:].ap[0][0], P], [ncols, CH], [1, PH]])
                nc.tensor.matmul(
                    ps2[:, h * CH * PH:(h + 1) * CH * PH],
                    wxf_t[:, k * NWP:k * NWP + NWP], mov)
            if k % 2 == 0:
                nc.vector.tensor_copy(st[:], ps2[:])
            else:
                nc.scalar.copy(st[:], ps2[:])
            nc.sync.dma_start(feats[k], st[:])

    nc.compile()
    return nc


def _build_program(C, K, MW, KG=32, P=128, s1_dtype=None, out_f16=None):
    s1_dtype = s1_dtype or S1_DTYPE
    out_f16 = OUT_F16 if out_f16 is None else out_f16
    NW = 2 * MW
    G = K // KG
    assert K % KG == 0
    ncols = KG * PH
    out_dt = F16 if out_f16 else F32

    nc = bacc.Bacc("TRN2", target_bir_lowering=False, debug=False,
                   enable_asserts=True, num_devices=1)

    img = nc.dram_tensor("img", [P, C * P], s1_dtype, kind="ExternalInput").ap()
    wy = nc.dram_tensor("wy", [P, K * PH], s1_dtype, kind="ExternalInput").ap()
    wxf = nc.dram_tensor("wxf", [P, K * NW], F16, kind="ExternalInput").ap()
    feats = nc.dram_tensor("feats", [K, 2 * C * PH * MW], out_dt,
                           kind="ExternalOutput").ap()

    blk = PH * MW
    # stage-2 PSUM: per i a 128-col (512B-aligned) pair block holding
    # [dir0 row i (MW cols) | dir1 row PH-1-i (MW cols)]; 122 cols fit one
    # 512B-aligned window so a single N=2*MW matmul never crosses a bank.
    assert 2 * MW <= 128
    ps2_cols = PH * 128

    with tile.TileContext(nc) as tc, ExitStack() as ctx:
        const_pool = ctx.enter_context(tc.tile_pool(name="const", bufs=1))
        s_pool = ctx.enter_context(tc.tile_pool(name="sg", bufs=2))
        tmp_pool = ctx.enter_context(tc.tile_pool(name="tmp", bufs=4))
        wxf_pool = ctx.enter_context(tc.tile_pool(name="wxf", bufs=1))
        out_pool = ctx.enter_context(tc.tile_pool(name="outt", bufs=4))
        ps1_pool = ctx.enter_context(tc.tile_pool(name="ps1", bufs=4, space="PSUM"))
        ps2_pool = ctx.enter_context(tc.tile_pool(name="ps2", bufs=2, space="PSUM"))

        img_t = const_pool.tile([P, C * P], s1_dtype)
        wy_t = const_pool.tile([P, K * PH], s1_dtype)
        nc.sync.dma_start(img_t[:], img)
        nc.sync.dma_start(wy_t[:], wy)

        for g in range(G):
            # S layout is ki-major (stationary-contiguous): free index =
            # ki*C + c. PSUM->SBUF evacuation is two-step: contiguous CAST
            # (DVE/ACT) to a small fp16 tmp, then a strided fp16 scatter on
            # the otherwise-idle GpSimd engine.
            sg = s_pool.tile([P, ncols * C], F16, tag="sg")
            wxf_t = wxf_pool.tile([P, KG * NW], F16, tag="wxf")
            nc.sync.dma_start(wxf_t[:], wxf[:, g * KG * NW:(g + 1) * KG * NW])
            for c in range(C):
                ps1 = ps1_pool.tile([P, ncols], F32, tag="ps1")
                nc.tensor.matmul(
                    ps1[:],
                    img_t[:, c * P:(c + 1) * P],
                    wy_t[:, g * ncols:(g + 1) * ncols],
                )
                tmp = tmp_pool.tile([P, ncols], F16, tag="tmp")
                if c % 3 == 2:
                    nc.scalar.copy(tmp[:], ps1[:])
                else:
                    nc.vector.tensor_copy(tmp[:], ps1[:])
                nc.gpsimd.tensor_copy(sg[:, c::C], tmp[:])

            for kl in range(KG):
                k = g * KG + kl
                ps2 = ps2_pool.tile([C, ps2_cols], F32, tag="ps2")
                for i in range(PH):
                    lhsT = sg[:, (kl * PH + i) * C:(kl * PH + i + 1) * C]
                    nc.tensor.matmul(
                        ps2[:, i * 128:i * 128 + NW],
                        lhsT, wxf_t[:, kl * NW:(kl + 1) * NW])

                # unshuffle pair blocks -> outt [c, (d, i, j)]
                outt = out_pool.tile([C, 2 * blk], out_dt, tag="outt")
                base = ps2[:]
                in0 = bass.AP(base.tensor, base.offset,
                              [[ps2_cols, C], [128, PH], [1, MW]])
                in1 = bass.AP(base.tensor, base.offset + 7 * 128 + MW,
                              [[ps2_cols, C], [-128, PH], [1, MW]])
                if kl % 3 == 2:
                    nc.scalar.copy(outt[:, :blk], in0)
                    nc.scalar.copy(outt[:, blk:], in1)
                else:
                    nc.vector.tensor_copy(outt[:, :blk], in0)
                    nc.vector.tensor_copy(outt[:, blk:], in1)
                # DRAM box block order is (d, c, ij); SBUF iterates (c, d, ij)
                box = bass.AP(feats.tensor, k * 2 * C * blk,
                              [[blk, C], [C * blk, 2], [1, blk]])
                nc.sync.dma_start(box, outt[:])

    nc.compile()
    return nc


_PROGRAM_CACHE = {}


def _get_program(C, K, MW):
    key = (C, K, MW)
    if key not in _PROGRAM_CACHE:
        _PROGRAM_CACHE[key] = _build_program_v4(C, K, MW)
    return _PROGRAM_CACHE[key]


# ----------------------------------------------------------------------------
# entry point
# ----------------------------------------------------------------------------

def kernel(x, boxes, _run_kwargs=None):
    x = np.asarray(x, dtype=np.float32)
    boxes = np.asarray(boxes, dtype=np.float32)
    N, C, H, W = x.shape
    K = boxes.shape[1]
    assert N == N_CORES and H == 128 and W == 128 and C == 128

    MW = _max_width(boxes)
    nc = _get_program(C, K, MW)

    s1_np = np.float16 if S1_DTYPE == F16 else np.float32
    in_maps = []
    per_image = []
    for n in range(N):
        WY, WXF, width, tall_idx = _build_image_weights(boxes[n], H, W, MW)
        per_image.append((width, tall_idx))
        img = np.ascontiguousarray(
            x[n].transpose(1, 0, 2).reshape(H, C * W))  # [h, (c, w)]
        WXFP = np.zeros((W, K * 128), np.float16)
        for k in range(K):
            WXFP[:, k * 128:k * 128 + 2 * MW] = WXF[:, k * 2 * MW:(k + 1) * 2 * MW]
        in_maps.append({
            "img": img.astype(s1_np),
            "wy": WY.astype(s1_np),
            "wxf": WXFP,
        })

    res = bass_utils.run_bass_kernel_spmd(
        nc, in_maps, core_ids=list(range(N_CORES)), **(_run_kwargs or {}))

    feats = np.empty((N, K, 2, C, PH, MW), np.float32)
    widths = np.empty((N, K, 2), np.float32)
    CH = C // 2
    for n in range(N):
        # device scratch layout: [k, jd, (h, c', i)] with c = h*CH + c';
        # jd = d*MW + j (rows 2*MW..127 are padding); dir1 row is PH-1-i
        s = res.results[n]["feats"].reshape(K, 128, 2, CH, PH)
        s = s[:, :2 * MW].reshape(K, 2, MW, 2, CH, PH)
        t = s.transpose(0, 1, 3, 4, 5, 2)  # (k, d, h, c', i, j)
        t = np.concatenate([t[:, :1], t[:, 1:, :, :, ::-1, :]], axis=1)
        feats[n] = t.reshape(K, 2, C, PH, MW).astype(np.float32)
        width, tall_idx = per_image[n]
        for k in tall_idx:
            feats[n, k] = _tall_feats(x[n], boxes[n], k, H, W, MW)
        widths[n] = width.astype(np.float32)[:, None]
    kernel.last_result = res
    return feats, widths


# revision 19
# speedup vs baseline: 1.2031x; 1.2031x over previous
"""Trainium2 Bass kernel for nn_BidirectionalBoxPool.

Contract: kernel(x, boxes) takes FULL inputs (x: (8,128,128,128) f32,
boxes: (8,64,4) f32) and returns (feats, widths) matching the reference:
feats (8, 64, 2, 128, 8, MW) f32, widths (8, 64, 2) f32, with MW the
data-dependent max pooled width.

Strategy: data-parallel over the batch axis N — core n handles image n.

Math per image: grid_sample with a per-box separable bilinear grid, so
  feats[k,d,c,i,j] = sum_h sum_w img[c,h,w] * wy_k[h,i] * wx_k[w,j']
with the dir-1 grid being an exact (i,j)-flip of dir-0 within each box's
valid width. Host code (numpy) replicates the reference's fp32 grid math
exactly and bakes it into two small weight tensors per image:
  WY  [h=128, K*8]     y-interp weights (f32, fed to fp32r matmuls)
  WXF [w=128, K*2*MW]  x-interp weights, dir0 block + flipped dir1 block (f16)
Device program (SPMD identical across cores; all box data flows through
the weight tensors):
  stage 1: per channel c: PSUM[w,(k,i)] = img_c[h,w]^T @ WY   (fp32r)
           -> SBUF S[w, (k,i)*C+c] in fp16
  stage 2: per (k,i): PSUM2[c, cols] = S_ki[w,c]^T @ WXF_k    (fp16)
           two matmuls (dir0/dir1) into bank-aligned PSUM halves
  out: per box, copy PSUM2 -> SBUF [c,(d,i,j)] and DMA to DRAM feats[k].
Tall boxes (bh > bw, ~7%, width<=16) have their grid transposed relative
to the wide layout; they are zeroed on device (zero weights) and patched
in exactly on host, as is the `widths` output.
"""

from contextlib import ExitStack

import numpy as np

import concourse.bass as bass
import concourse.tile as tile
from concourse import bacc, bass_utils, mybir

F32 = mybir.dt.float32
F32R = mybir.dt.float32r
F16 = mybir.dt.float16

PH = 8
N_CORES = 8
NPF32 = np.float32


# ----------------------------------------------------------------------------
# host-side weight construction (replicates reference fp32 grid math)
# ----------------------------------------------------------------------------

def _box_meta(boxes):
    b = boxes.astype(NPF32)
    xmin, ymin, xmax, ymax = b[:, 0], b[:, 1], b[:, 2], b[:, 3]
    valid = ~((xmin == 0) & (ymin == 0) & (xmax == 0) & (ymax == 0))
    one = NPF32(1.0)
    bw = np.where(valid, (xmax - xmin).astype(NPF32), one).astype(NPF32)
    bh = np.where(valid, (ymax - ymin).astype(NPF32), one).astype(NPF32)
    wide = bw > bh
    ratio = np.where(wide, (bw / bh).astype(NPF32),
                     (bh / bw).astype(NPF32)).astype(NPF32)
    width = np.ceil((ratio * NPF32(PH)).astype(NPF32)).astype(np.int32)
    width = np.where(valid, width, 0)
    wf = np.maximum(width, 2).astype(NPF32)
    return valid, wide, width, wf, bw, bh, xmin.astype(NPF32), ymin.astype(NPF32)


def _max_width(boxes_all):
    b = np.asarray(boxes_all, dtype=np.float64)
    valid = ~np.all(b == 0, axis=-1)
    bw = np.where(valid, b[..., 2] - b[..., 0], 1.0)
    bh = np.where(valid, b[..., 3] - b[..., 1], 1.0)
    ratio = np.where(bw > bh, bw / bh, bh / bw)
    ratio = np.where(valid, ratio, 0.0)
    return int(np.ceil(ratio.max() * PH))


def _grid_wide(xmin, ymin, bw, bh, wf, W, H, ii, jj):
    gx = ((xmin + (jj * bw / (wf - NPF32(1.0))).astype(NPF32)).astype(NPF32)
          - NPF32(W / 2)) / NPF32(W / 2)
    gy = ((ymin + (ii * bh / NPF32(PH - 1.0)).astype(NPF32)).astype(NPF32)
          - NPF32(H / 2)) / NPF32(H / 2)
    return gx.astype(NPF32), gy.astype(NPF32)


def _grid_tall(xmin, ymin, bw, bh, wf, W, H, ii, jj):
    gx = ((xmin + (ii * bw / NPF32(PH - 1.0)).astype(NPF32)).astype(NPF32)
          - NPF32(W / 2)) / NPF32(W / 2)
    gy = ((ymin + ((wf - jj) * bh / (wf - NPF32(1.0))).astype(NPF32)).astype(NPF32)
          - NPF32(H / 2)) / NPF32(H / 2)
    return gx.astype(NPF32), gy.astype(NPF32)


def _taps(g, n):
    g = g.astype(NPF32)
    pos = ((g + NPF32(1.0)) * NPF32(n) - NPF32(1.0)) * NPF32(0.5)
    pos64 = pos.astype(np.float64)
    i0 = np.floor(pos64).astype(np.int64)
    f = pos64 - i0
    w0 = np.where((i0 >= 0) & (i0 <= n - 1), 1.0 - f, 0.0)
    w1 = np.where((i0 + 1 >= 0) & (i0 + 1 <= n - 1), f, 0.0)
    return i0, w0, w1


def _build_image_weights(boxes, H, W, MW):
    K = boxes.shape[0]
    valid, wide, width, wf, bw, bh, xmin, ymin = _box_meta(boxes)
    WY = np.zeros((H, K * PH), np.float64)
    WXF = np.zeros((W, K * 2 * MW), np.float64)
    tall_idx = []
    ii = np.arange(PH, dtype=NPF32)
    for k in range(K):
        if not valid[k]:
            continue
        if not wide[k]:
            tall_idx.append(k)
            continue
        wk = int(width[k])
        jj = np.arange(wk, dtype=NPF32)
        gx, gy = _grid_wide(xmin[k], ymin[k], bw[k], bh[k], wf[k], W, H, ii, jj)
        y0, wy0, wy1 = _taps(gy, H)
        for i in range(PH):
            col = k * PH + i
            if wy0[i] != 0.0:
                WY[y0[i], col] += wy0[i]
            if wy1[i] != 0.0:
                WY[y0[i] + 1, col] += wy1[i]
        x0, wx0, wx1 = _taps(gx, W)
        base = k * 2 * MW
        for j in range(min(wk, MW)):
            if wx0[j] != 0.0:
                WXF[x0[j], base + j] += wx0[j]
            if wx1[j] != 0.0:
                WXF[x0[j] + 1, base + j] += wx1[j]
            jr = wk - 1 - j
            if wx0[jr] != 0.0:
                WXF[x0[jr], base + MW + j] += wx0[jr]
            if wx1[jr] != 0.0:
                WXF[x0[jr] + 1, base + MW + j] += wx1[jr]
    return WY.astype(NPF32), WXF.astype(NPF32), width, tall_idx


def _tall_feats(img, boxes, k, H, W, MW):
    valid, wide, width, wf, bw, bh, xmin, ymin = _box_meta(boxes)
    C = img.shape[0]
    wk = int(width[k])
    out = np.zeros((2, C, PH, MW), NPF32)
    ii = np.arange(PH, dtype=NPF32)[:, None]
    jj = np.arange(wk, dtype=NPF32)[None, :]
    gx, gy = _grid_tall(xmin[k], ymin[k], bw[k], bh[k], wf[k], W, H, ii, jj)
    gx = np.broadcast_to(gx, (PH, wk))
    gy = np.broadcast_to(gy, (PH, wk))
    x0, wx0, wx1 = _taps(gx, W)
    y0, wy0, wy1 = _taps(gy, H)
    imgf = img.astype(np.float64)

    def gat(yc, xc, m):
        yi = np.clip(yc, 0, H - 1)
        xi = np.clip(xc, 0, W - 1)
        return imgf[:, yi, xi] * m

    s = (gat(y0, x0, wy0 * wx0) + gat(y0, x0 + 1, wy0 * wx1)
         + gat(y0 + 1, x0, wy1 * wx0) + gat(y0 + 1, x0 + 1, wy1 * wx1))
    wcl = min(wk, MW)
    out[0, :, :, :wcl] = s[:, :, :wcl].astype(NPF32)
    out[1, :, :, :wcl] = s[:, ::-1, ::-1][:, :, :wcl].astype(NPF32)
    return out


# ----------------------------------------------------------------------------
# device program
# ----------------------------------------------------------------------------

S1_DTYPE = F16  # stage-1 matmul dtype: F16 (fast) or F32R (higher precision)
OUT_F16 = True  # device writes fp16 feats; host upcasts


def _build_program_v4(C, K, MW, KG=None, P=128):
    """v4: stage-2 swapped — stationary = WXF_k (contiguous, 1 LDW/box),
    moving = S block (c-major sg, runs-of-8 strided AP, full rate).
    Output goes to a DRAM scratch [k, jd, (h, c', i)] (fp16, 2KB/partition
    DMA chunks); the final (k,d,c,i,j) permutation happens on host."""
    KG = KG or 32  # moving-AP c-stride is ncols*2B; keep at 512B (KG=32)
    NW = 2 * MW
    NWP = 128  # wxf padded to 128 cols/box so DMA tiles have 128 partitions
    assert NW <= NWP
    G = K // KG
    assert K % KG == 0 and C % 2 == 0
    ncols = KG * PH
    CH = C // 2  # c-half per N=CH*PH matmul

    nc = bacc.Bacc("TRN2", target_bir_lowering=False, debug=False,
                   enable_asserts=True, num_devices=1)

    img = nc.dram_tensor("img", [P, C * P], F16, kind="ExternalInput").ap()
    wy = nc.dram_tensor("wy", [P, K * PH], F16, kind="ExternalInput").ap()
    wxf = nc.dram_tensor("wxf", [P, K * NWP], F16, kind="ExternalInput").ap()
    feats = nc.dram_tensor("feats", [K, NWP * C * PH], F16,
                           kind="ExternalOutput").ap()

    with tile.TileContext(nc) as tc, ExitStack() as ctx:
        const_pool = ctx.enter_context(tc.tile_pool(name="const", bufs=1))
        s_pool = ctx.enter_context(tc.tile_pool(name="sg", bufs=2))
        st_pool = ctx.enter_context(tc.tile_pool(name="st", bufs=4))
        ps1_pool = ctx.enter_context(tc.tile_pool(name="ps1", bufs=3, space="PSUM"))
        ps2_pool = ctx.enter_context(tc.tile_pool(name="ps2", bufs=2, space="PSUM"))

        img_t = const_pool.tile([P, C * P], F16)
        wy_t = const_pool.tile([P, K * PH], F16)
        wxf_t = const_pool.tile([P, K * NWP], F16)
        nc.sync.dma_start(wy_t[:], wy)
        nc.sync.dma_start(wxf_t[:], wxf)
        CCH = min(16, C)  # img DMA chunk (channels) so stage 1 starts early
        for cc in range(0, C, CCH):
            nc.sync.dma_start(img_t[:, cc * P:(cc + CCH) * P],
                              img[:, cc * P:(cc + CCH) * P])

        for g in range(G):
            # stage 1: c-major S (free index = c*ncols + ki); two channels
            # share one ps1 tile so casts move 2*ncols columns at a time
            sg = s_pool.tile([P, C * ncols], F16, tag="sg")
            for c2 in range(C // 2):
                ps1 = ps1_pool.tile([P, 2 * ncols], F32, tag="ps1")
                for h in range(2):
                    c = 2 * c2 + h
                    nc.tensor.matmul(
                        ps1[:, h * ncols:(h + 1) * ncols],
                        img_t[:, c * P:(c + 1) * P],
                        wy_t[:, g * ncols:(g + 1) * ncols],
                    )
                if c2 % 2 == 0:
                    nc.vector.tensor_copy(
                        sg[:, 2 * c2 * ncols:(2 * c2 + 2) * ncols], ps1[:])
                else:
                    nc.scalar.copy(
                        sg[:, 2 * c2 * ncols:(2 * c2 + 2) * ncols], ps1[:])

            # stage 2 (swapped): per box, one LDW + 2 matmuls N=CH*PH
            for kl in range(KG):
                k = g * KG + kl
                st = st_pool.tile([NWP, C * PH], F16, tag="st")
                ps2 = ps2_pool.tile([NWP, C * PH], F32, tag="ps2")
                for h in range(2):
                    mov = bass.AP(sg.tensor,
                                  sg[:].offset + h * CH * ncols + kl * PH,
                                  [[sg[:].ap[0][0], P], [ncols, CH], [1, PH]])
                    nc.tensor.matmul(
                        ps2[:, h * CH * PH:(h + 1) * CH * PH],
                        wxf_t[:, k * NWP:k * NWP + NWP], mov)
                if kl % 2 == 0:
                    nc.vector.tensor_copy(st[:], ps2[:])
                else:
                    nc.scalar.copy(st[:], ps2[:])
                nc.sync.dma_start(feats[k], st[:])

    nc.compile()
    return nc


def _build_program(C, K, MW, KG=32, P=128, s1_dtype=None, out_f16=None):
    s1_dtype = s1_dtype or S1_DTYPE
    out_f16 = OUT_F16 if out_f16 is None else out_f16
    NW = 2 * MW
    G = K // KG
    assert K % KG == 0
    ncols = KG * PH
    out_dt = F16 if out_f16 else F32

    nc = bacc.Bacc("TRN2", target_bir_lowering=False, debug=False,
                   enable_asserts=True, num_devices=1)

    img = nc.dram_tensor("img", [P, C * P], s1_dtype, kind="ExternalInput").ap()
    wy = nc.dram_tensor("wy", [P, K * PH], s1_dtype, kind="ExternalInput").ap()
    wxf = nc.dram_tensor("wxf", [P, K * NW], F16, kind="ExternalInput").ap()
    feats = nc.dram_tensor("feats", [K, 2 * C * PH * MW], out_dt,
                           kind="ExternalOutput").ap()

    blk = PH * MW
    # stage-2 PSUM: per i a 128-col (512B-aligned) pair block holding
    # [dir0 row i (MW cols) | dir1 row PH-1-i (MW cols)]; 122 cols fit one
    # 512B-aligned window so a single N=2*MW matmul never crosses a bank.
    assert 2 * MW <= 128
    ps2_cols = PH * 128

    with tile.TileContext(nc) as tc, ExitStack() as ctx:
        const_pool = ctx.enter_context(tc.tile_pool(name="const", bufs=1))
        s_pool = ctx.enter_context(tc.tile_pool(name="sg", bufs=2))
        tmp_pool = ctx.enter_context(tc.tile_pool(name="tmp", bufs=4))
        wxf_pool = ctx.enter_context(tc.tile_pool(name="wxf", bufs=1))
        out_pool = ctx.enter_context(tc.tile_pool(name="outt", bufs=4))
        ps1_pool = ctx.enter_context(tc.tile_pool(name="ps1", bufs=4, space="PSUM"))
        ps2_pool = ctx.enter_context(tc.tile_pool(name="ps2", bufs=2, space="PSUM"))

        img_t = const_pool.tile([P, C * P], s1_dtype)
        wy_t = const_pool.tile([P, K * PH], s1_dtype)
        nc.sync.dma_start(img_t[:], img)
        nc.sync.dma_start(wy_t[:], wy)

        for g in range(G):
            # S layout is ki-major (stationary-contiguous): free index =
            # ki*C + c. PSUM->SBUF evacuation is two-step: contiguous CAST
            # (DVE/ACT) to a small fp16 tmp, then a strided fp16 scatter on
            # the otherwise-idle GpSimd engine.
            sg = s_pool.tile([P, ncols * C], F16, tag="sg")
            wxf_t = wxf_pool.tile([P, KG * NW], F16, tag="wxf")
            nc.sync.dma_start(wxf_t[:], wxf[:, g * KG * NW:(g + 1) * KG * NW])
            for c in range(C):
                ps1 = ps1_pool.tile([P, ncols], F32, tag="ps1")
                nc.tensor.matmul(
                    ps1[:],
                    img_t[:, c * P:(c + 1) * P],
                    wy_t[:, g * ncols:(g + 1) * ncols],
                )
                tmp = tmp_pool.tile([P, ncols], F16, tag="tmp")
                if c % 3 == 2:
                    nc.scalar.copy(tmp[:], ps1[:])
                else:
                    nc.vector.tensor_copy(tmp[:], ps1[:])
                nc.gpsimd.tensor_copy(sg[:, c::C], tmp[:])

            for kl in range(KG):
                k = g * KG + kl
                ps2 = ps2_pool.tile([C, ps2_cols], F32, tag="ps2")
                for i in range(PH):
                    lhsT = sg[:, (kl * PH + i) * C:(kl * PH + i + 1) * C]
                    nc.tensor.matmul(
                        ps2[:, i * 128:i * 128 + NW],
                        lhsT, wxf_t[:, kl * NW:(kl + 1) * NW])

                # unshuffle pair blocks -> outt [c, (d, i, j)]
                outt = out_pool.tile([C, 2 * blk], out_dt, tag="outt")
                base = ps2[:]
                in0 = bass.AP(base.tensor, base.offset,
                              [[ps2_cols, C], [128, PH], [1, MW]])
                in1 = bass.AP(base.tensor, base.offset + 7 * 128 + MW,
                              [[ps2_cols, C], [-128, PH], [1, MW]])
                if kl % 3 == 2:
                    nc.scalar.copy(outt[:, :blk], in0)
                    nc.scalar.copy(outt[:, blk:], in1)
                else:
                    nc.vector.tensor_copy(outt[:, :blk], in0)
                    nc.vector.tensor_copy(outt[:, blk:], in1)
                # DRAM box block order is (d, c, ij); SBUF iterates (c, d, ij)
                box = bass.AP(feats.tensor, k * 2 * C * blk,
                              [[blk, C], [C * blk, 2], [1, blk]])
                nc.sync.dma_start(box, outt[:])

    nc.compile()
    return nc


_PROGRAM_CACHE = {}


def _get_program(C, K, MW):
    key = (C, K, MW)
    if key not in _PROGRAM_CACHE:
        _PROGRAM_CACHE[key] = _build_program_v4(C, K, MW)
    return _PROGRAM_CACHE[key]


# ----------------------------------------------------------------------------
# entry point
# ----------------------------------------------------------------------------

def kernel(x, boxes, _run_kwargs=None):
    x = np.asarray(x, dtype=np.float32)
    boxes = np.asarray(boxes, dtype=np.float32)
    N, C, H, W = x.shape
    K = boxes.shape[1]
    assert N == N_CORES and H == 128 and W == 128 and C == 128

    MW = _max_width(boxes)
    nc = _get_program(C, K, MW)

    s1_np = np.float16 if S1_DTYPE == F16 else np.float32
    in_maps = []
    per_image = []
    for n in range(N):
        WY, WXF, width, tall_idx = _build_image_weights(boxes[n], H, W, MW)
        per_image.append((width, tall_idx))
        img = np.ascontiguousarray(
            x[n].transpose(1, 0, 2).reshape(H, C * W))  # [h, (c, w)]
        WXFP = np.zeros((W, K * 128), np.float16)
        for k in range(K):
            WXFP[:, k * 128:k * 128 + 2 * MW] = WXF[:, k * 2 * MW:(k + 1) * 2 * MW]
        in_maps.append({
            "img": img.astype(s1_np),
            "wy": WY.astype(s1_np),
            "wxf": WXFP,
        })

    res = bass_utils.run_bass_kernel_spmd(
        nc, in_maps, core_ids=list(range(N_CORES)), **(_run_kwargs or {}))

    feats = np.empty((N, K, 2, C, PH, MW), np.float32)
    widths = np.empty((N, K, 2), np.float32)
    CH = C // 2
    for n in range(N):
        # device scratch layout: [k, jd, (h, c', i)] with c = h*CH + c';
        # jd = d*MW + j (rows 2*MW..127 are padding); dir1 row is PH-1-i
        s = res.results[n]["feats"].reshape(K, 128, 2, CH, PH)
        s = s[:, :2 * MW].reshape(K, 2, MW, 2, CH, PH)
        t = s.transpose(0, 1, 3, 4, 5, 2)  # (k, d, h, c', i, j)
        t = np.concatenate([t[:, :1], t[:, 1:, :, :, ::-1, :]], axis=1)
        feats[n] = t.reshape(K, 2, C, PH, MW).astype(np.float32)
        width, tall_idx = per_image[n]
        for k in tall_idx:
            feats[n, k] = _tall_feats(x[n], boxes[n], k, H, W, MW)
        widths[n] = width.astype(np.float32)[:, None]
    kernel.last_result = res
    return feats, widths


# revision 21
# speedup vs baseline: 1.3574x; 1.1282x over previous
"""Trainium2 Bass kernel for nn_BidirectionalBoxPool.

Contract: kernel(x, boxes) takes FULL inputs (x: (8,128,128,128) f32,
boxes: (8,64,4) f32) and returns (feats, widths) matching the reference:
feats (8, 64, 2, 128, 8, MW) f32, widths (8, 64, 2) f32, with MW the
data-dependent max pooled width.

Strategy: data-parallel over the batch axis N — core n handles image n.

Math per image: grid_sample with a per-box separable bilinear grid, so
  feats[k,d,c,i,j] = sum_h sum_w img[c,h,w] * wy_k[h,i] * wx_k[w,j']
with the dir-1 grid being an exact (i,j)-flip of dir-0 within each box's
valid width. Host code (numpy) replicates the reference's fp32 grid math
exactly and bakes it into two small weight tensors per image:
  WY  [h=128, K*8]     y-interp weights (f32, fed to fp32r matmuls)
  WXF [w=128, K*2*MW]  x-interp weights, dir0 block + flipped dir1 block (f16)
Device program (SPMD identical across cores; all box data flows through
the weight tensors):
  stage 1: per channel c: PSUM[w,(k,i)] = img_c[h,w]^T @ WY   (fp32r)
           -> SBUF S[w, (k,i)*C+c] in fp16
  stage 2: per (k,i): PSUM2[c, cols] = S_ki[w,c]^T @ WXF_k    (fp16)
           two matmuls (dir0/dir1) into bank-aligned PSUM halves
  out: per box, copy PSUM2 -> SBUF [c,(d,i,j)] and DMA to DRAM feats[k].
Tall boxes (bh > bw, ~7%, width<=16) have their grid transposed relative
to the wide layout; they are zeroed on device (zero weights) and patched
in exactly on host, as is the `widths` output.
"""

from contextlib import ExitStack

import numpy as np

import concourse.bass as bass
import concourse.tile as tile
from concourse import bacc, bass_utils, mybir

F32 = mybir.dt.float32
F32R = mybir.dt.float32r
F16 = mybir.dt.float16

PH = 8
N_CORES = 8
NPF32 = np.float32


# ----------------------------------------------------------------------------
# host-side weight construction (replicates reference fp32 grid math)
# ----------------------------------------------------------------------------

def _box_meta(boxes):
    b = boxes.astype(NPF32)
    xmin, ymin, xmax, ymax = b[:, 0], b[:, 1], b[:, 2], b[:, 3]
    valid = ~((xmin == 0) & (ymin == 0) & (xmax == 0) & (ymax == 0))
    one = NPF32(1.0)
    bw = np.where(valid, (xmax - xmin).astype(NPF32), one).astype(NPF32)
    bh = np.where(valid, (ymax - ymin).astype(NPF32), one).astype(NPF32)
    wide = bw > bh
    ratio = np.where(wide, (bw / bh).astype(NPF32),
                     (bh / bw).astype(NPF32)).astype(NPF32)
    width = np.ceil((ratio * NPF32(PH)).astype(NPF32)).astype(np.int32)
    width = np.where(valid, width, 0)
    wf = np.maximum(width, 2).astype(NPF32)
    return valid, wide, width, wf, bw, bh, xmin.astype(NPF32), ymin.astype(NPF32)


def _max_width(boxes_all):
    b = np.asarray(boxes_all, dtype=np.float64)
    valid = ~np.all(b == 0, axis=-1)
    bw = np.where(valid, b[..., 2] - b[..., 0], 1.0)
    bh = np.where(valid, b[..., 3] - b[..., 1], 1.0)
    ratio = np.where(bw > bh, bw / bh, bh / bw)
    ratio = np.where(valid, ratio, 0.0)
    return int(np.ceil(ratio.max() * PH))


def _grid_wide(xmin, ymin, bw, bh, wf, W, H, ii, jj):
    gx = ((xmin + (jj * bw / (wf - NPF32(1.0))).astype(NPF32)).astype(NPF32)
          - NPF32(W / 2)) / NPF32(W / 2)
    gy = ((ymin + (ii * bh / NPF32(PH - 1.0)).astype(NPF32)).astype(NPF32)
          - NPF32(H / 2)) / NPF32(H / 2)
    return gx.astype(NPF32), gy.astype(NPF32)


def _grid_tall(xmin, ymin, bw, bh, wf, W, H, ii, jj):
    gx = ((xmin + (ii * bw / NPF32(PH - 1.0)).astype(NPF32)).astype(NPF32)
          - NPF32(W / 2)) / NPF32(W / 2)
    gy = ((ymin + ((wf - jj) * bh / (wf - NPF32(1.0))).astype(NPF32)).astype(NPF32)
          - NPF32(H / 2)) / NPF32(H / 2)
    return gx.astype(NPF32), gy.astype(NPF32)


def _taps(g, n):
    g = g.astype(NPF32)
    pos = ((g + NPF32(1.0)) * NPF32(n) - NPF32(1.0)) * NPF32(0.5)
    pos64 = pos.astype(np.float64)
    i0 = np.floor(pos64).astype(np.int64)
    f = pos64 - i0
    w0 = np.where((i0 >= 0) & (i0 <= n - 1), 1.0 - f, 0.0)
    w1 = np.where((i0 + 1 >= 0) & (i0 + 1 <= n - 1), f, 0.0)
    return i0, w0, w1


def _build_image_weights(boxes, H, W, MW):
    K = boxes.shape[0]
    valid, wide, width, wf, bw, bh, xmin, ymin = _box_meta(boxes)
    WY = np.zeros((H, K * PH), np.float64)
    WXF = np.zeros((W, K * 2 * MW), np.float64)
    tall_idx = []
    ii = np.arange(PH, dtype=NPF32)
    for k in range(K):
        if not valid[k]:
            continue
        if not wide[k]:
            tall_idx.append(k)
            continue
        wk = int(width[k])
        jj = np.arange(wk, dtype=NPF32)
        gx, gy = _grid_wide(xmin[k], ymin[k], bw[k], bh[k], wf[k], W, H, ii, jj)
        y0, wy0, wy1 = _taps(gy, H)
        for i in range(PH):
            col = k * PH + i
            if wy0[i] != 0.0:
                WY[y0[i], col] += wy0[i]
            if wy1[i] != 0.0:
                WY[y0[i] + 1, col] += wy1[i]
        x0, wx0, wx1 = _taps(gx, W)
        base = k * 2 * MW
        for j in range(min(wk, MW)):
            if wx0[j] != 0.0:
                WXF[x0[j], base + j] += wx0[j]
            if wx1[j] != 0.0:
                WXF[x0[j] + 1, base + j] += wx1[j]
            jr = wk - 1 - j
            if wx0[jr] != 0.0:
                WXF[x0[jr], base + MW + j] += wx0[jr]
            if wx1[jr] != 0.0:
                WXF[x0[jr] + 1, base + MW + j] += wx1[jr]
    return WY.astype(NPF32), WXF.astype(NPF32), width, tall_idx


def _tall_feats(img, boxes, k, H, W, MW):
    valid, wide, width, wf, bw, bh, xmin, ymin = _box_meta(boxes)
    C = img.shape[0]
    wk = int(width[k])
    out = np.zeros((2, C, PH, MW), NPF32)
    ii = np.arange(PH, dtype=NPF32)[:, None]
    jj = np.arange(wk, dtype=NPF32)[None, :]
    gx, gy = _grid_tall(xmin[k], ymin[k], bw[k], bh[k], wf[k], W, H, ii, jj)
    gx = np.broadcast_to(gx, (PH, wk))
    gy = np.broadcast_to(gy, (PH, wk))
    x0, wx0, wx1 = _taps(gx, W)
    y0, wy0, wy1 = _taps(gy, H)
    imgf = img.astype(np.float64)

    def gat(yc, xc, m):
        yi = np.clip(yc, 0, H - 1)
        xi = np.clip(xc, 0, W - 1)
        return imgf[:, yi, xi] * m

    s = (gat(y0, x0, wy0 * wx0) + gat(y0, x0 + 1, wy0 * wx1)
         + gat(y0 + 1, x0, wy1 * wx0) + gat(y0 + 1, x0 + 1, wy1 * wx1))
    wcl = min(wk, MW)
    out[0, :, :, :wcl] = s[:, :, :wcl].astype(NPF32)
    out[1, :, :, :wcl] = s[:, ::-1, ::-1][:, :, :wcl].astype(NPF32)
    return out


# ----------------------------------------------------------------------------
# device program
# ----------------------------------------------------------------------------

S1_DTYPE = F16  # stage-1 matmul dtype: F16 (fast) or F32R (higher precision)
OUT_F16 = True  # device writes fp16 feats; host upcasts


def _build_program_v4(C, K, MW, KG=None, P=128):
    """v4: stage-2 swapped — stationary = WXF_k (contiguous, 1 LDW/box),
    moving = S block (c-major sg, runs-of-8 strided AP, full rate).
    Output goes to a DRAM scratch [k, jd, (h, c', i)] (fp16, 2KB/partition
    DMA chunks); the final (k,d,c,i,j) permutation happens on host."""
    KG = KG or 32  # moving-AP c-stride is ncols*2B; keep at 512B (KG=32)
    NW = 2 * MW
    NWP = 128  # wxf padded to 128 cols/box so DMA tiles have 128 partitions
    assert NW <= NWP
    G = K // KG
    assert K % KG == 0 and C % 2 == 0
    ncols = KG * PH
    CH = C // 2  # c-half per N=CH*PH matmul

    nc = bacc.Bacc("TRN2", target_bir_lowering=False, debug=False,
                   enable_asserts=True, num_devices=1)

    img = nc.dram_tensor("img", [P, C * P], F16, kind="ExternalInput").ap()
    wy = nc.dram_tensor("wy", [P, K * PH], F16, kind="ExternalInput").ap()
    wxf = nc.dram_tensor("wxf", [P, K * NWP], F16, kind="ExternalInput").ap()
    feats = nc.dram_tensor("feats", [K, NWP * C * PH], F16,
                           kind="ExternalOutput").ap()

    with tile.TileContext(nc) as tc, ExitStack() as ctx:
        const_pool = ctx.enter_context(tc.tile_pool(name="const", bufs=1))
        s_pool = ctx.enter_context(tc.tile_pool(name="sg", bufs=2))
        st_pool = ctx.enter_context(tc.tile_pool(name="st", bufs=4))
        ps1_pool = ctx.enter_context(tc.tile_pool(name="ps1", bufs=4, space="PSUM"))
        ps2_pool = ctx.enter_context(tc.tile_pool(name="ps2", bufs=2, space="PSUM"))

        img_t = const_pool.tile([P, C * P], F16)
        wy_t = const_pool.tile([P, K * PH], F16)
        wxf_t = const_pool.tile([P, K * NWP], F16)
        nc.sync.dma_start(wy_t[:], wy)
        CCH = min(16, C)  # img DMA chunk (channels) so stage 1 starts early
        for cc in range(0, C, CCH):
            nc.sync.dma_start(img_t[:, cc * P:(cc + CCH) * P],
                              img[:, cc * P:(cc + CCH) * P])
        nc.sync.dma_start(wxf_t[:], wxf)  # not needed until stage 2

        for g in range(G):
            # stage 1: c-major S (free index = c*ncols + ki); two channels
            # share one ps1 tile so casts move 2*ncols columns at a time
            sg = s_pool.tile([P, C * ncols], F16, tag="sg")
            for c2 in range(C // 2):
                ps1 = ps1_pool.tile([P, 2 * ncols], F32, tag="ps1")
                for h in range(2):
                    c = 2 * c2 + h
                    nc.tensor.matmul(
                        ps1[:, h * ncols:(h + 1) * ncols],
                        img_t[:, c * P:(c + 1) * P],
                        wy_t[:, g * ncols:(g + 1) * ncols],
                    )
                if c2 % 2 == 0:
                    nc.vector.tensor_copy(
                        sg[:, 2 * c2 * ncols:(2 * c2 + 2) * ncols], ps1[:])
                else:
                    nc.scalar.copy(
                        sg[:, 2 * c2 * ncols:(2 * c2 + 2) * ncols], ps1[:])

            # stage 2 (swapped): per box, one LDW + 2 matmuls N=CH*PH
            for kl in range(KG):
                k = g * KG + kl
                st = st_pool.tile([NWP, C * PH], F16, tag="st")
                ps2 = ps2_pool.tile([NWP, C * PH], F32, tag="ps2")
                for h in range(2):
                    mov = bass.AP(sg.tensor,
                                  sg[:].offset + h * CH * ncols + kl * PH,
                                  [[sg[:].ap[0][0], P], [ncols, CH], [1, PH]])
                    nc.tensor.matmul(
                        ps2[:, h * CH * PH:(h + 1) * CH * PH],
                        wxf_t[:, k * NWP:k * NWP + NWP], mov)
                if kl % 2 == 0:
                    nc.vector.tensor_copy(st[:], ps2[:])
                else:
                    nc.scalar.copy(st[:], ps2[:])
                nc.sync.dma_start(feats[k], st[:])

    nc.compile()
    return nc


def _build_program(C, K, MW, KG=32, P=128, s1_dtype=None, out_f16=None):
    s1_dtype = s1_dtype or S1_DTYPE
    out_f16 = OUT_F16 if out_f16 is None else out_f16
    NW = 2 * MW
    G = K // KG
    assert K % KG == 0
    ncols = KG * PH
    out_dt = F16 if out_f16 else F32

    nc = bacc.Bacc("TRN2", target_bir_lowering=False, debug=False,
                   enable_asserts=True, num_devices=1)

    img = nc.dram_tensor("img", [P, C * P], s1_dtype, kind="ExternalInput").ap()
    wy = nc.dram_tensor("wy", [P, K * PH], s1_dtype, kind="ExternalInput").ap()
    wxf = nc.dram_tensor("wxf", [P, K * NW], F16, kind="ExternalInput").ap()
    feats = nc.dram_tensor("feats", [K, 2 * C * PH * MW], out_dt,
                           kind="ExternalOutput").ap()

    blk = PH * MW
    # stage-2 PSUM: per i a 128-col (512B-aligned) pair block holding
    # [dir0 row i (MW cols) | dir1 row PH-1-i (MW cols)]; 122 cols fit one
    # 512B-aligned window so a single N=2*MW matmul never crosses a bank.
    assert 2 * MW <= 128
    ps2_cols = PH * 128

    with tile.TileContext(nc) as tc, ExitStack() as ctx:
        const_pool = ctx.enter_context(tc.tile_pool(name="const", bufs=1))
        s_pool = ctx.enter_context(tc.tile_pool(name="sg", bufs=2))
        tmp_pool = ctx.enter_context(tc.tile_pool(name="tmp", bufs=4))
        wxf_pool = ctx.enter_context(tc.tile_pool(name="wxf", bufs=1))
        out_pool = ctx.enter_context(tc.tile_pool(name="outt", bufs=4))
        ps1_pool = ctx.enter_context(tc.tile_pool(name="ps1", bufs=4, space="PSUM"))
        ps2_pool = ctx.enter_context(tc.tile_pool(name="ps2", bufs=2, space="PSUM"))

        img_t = const_pool.tile([P, C * P], s1_dtype)
        wy_t = const_pool.tile([P, K * PH], s1_dtype)
        nc.sync.dma_start(img_t[:], img)
        nc.sync.dma_start(wy_t[:], wy)

        for g in range(G):
            # S layout is ki-major (stationary-contiguous): free index =
            # ki*C + c. PSUM->SBUF evacuation is two-step: contiguous CAST
            # (DVE/ACT) to a small fp16 tmp, then a strided fp16 scatter on
            # the otherwise-idle GpSimd engine.
            sg = s_pool.tile([P, ncols * C], F16, tag="sg")
            wxf_t = wxf_pool.tile([P, KG * NW], F16, tag="wxf")
            nc.sync.dma_start(wxf_t[:], wxf[:, g * KG * NW:(g + 1) * KG * NW])
            for c in range(C):
                ps1 = ps1_pool.tile([P, ncols], F32, tag="ps1")
                nc.tensor.matmul(
                    ps1[:],
                    img_t[:, c * P:(c + 1) * P],
                    wy_t[:, g * ncols:(g + 1) * ncols],
                )
                tmp = tmp_pool.tile([P, ncols], F16, tag="tmp")
                if c % 3 == 2:
                    nc.scalar.copy(tmp[:], ps1[:])
                else:
                    nc.vector.tensor_copy(tmp[:], ps1[:])
                nc.gpsimd.tensor_copy(sg[:, c::C], tmp[:])

            for kl in range(KG):
                k = g * KG + kl
                ps2 = ps2_pool.tile([C, ps2_cols], F32, tag="ps2")
                for i in range(PH):
                    lhsT = sg[:, (kl * PH + i) * C:(kl * PH + i + 1) * C]
                    nc.tensor.matmul(
                        ps2[:, i * 128:i * 128 + NW],
                        lhsT, wxf_t[:, kl * NW:(kl + 1) * NW])

                # unshuffle pair blocks -> outt [c, (d, i, j)]
                outt = out_pool.tile([C, 2 * blk], out_dt, tag="outt")
                base = ps2[:]
                in0 = bass.AP(base.tensor, base.offset,
                              [[ps2_cols, C], [128, PH], [1, MW]])
                in1 = bass.AP(base.tensor, base.offset + 7 * 128 + MW,
                              [[ps2_cols, C], [-128, PH], [1, MW]])
                if kl % 3 == 2:
                    nc.scalar.copy(outt[:, :blk], in0)
                    nc.scalar.copy(outt[:, blk:], in1)
                else:
                    nc.vector.tensor_copy(outt[:, :blk], in0)
                    nc.vector.tensor_copy(outt[:, blk:], in1)
                # DRAM box block order is (d, c, ij); SBUF iterates (c, d, ij)
                box = bass.AP(feats.tensor, k * 2 * C * blk,
                              [[blk, C], [C * blk, 2], [1, blk]])
                nc.sync.dma_start(box, outt[:])

    nc.compile()
    return nc


_PROGRAM_CACHE = {}


def _get_program(C, K, MW):
    key = (C, K, MW)
    if key not in _PROGRAM_CACHE:
        _PROGRAM_CACHE[key] = _build_program_v4(C, K, MW)
    return _PROGRAM_CACHE[key]


# ----------------------------------------------------------------------------
# entry point
# ----------------------------------------------------------------------------

def kernel(x, boxes, _run_kwargs=None):
    x = np.asarray(x, dtype=np.float32)
    boxes = np.asarray(boxes, dtype=np.float32)
    N, C, H, W = x.shape
    K = boxes.shape[1]
    assert N == N_CORES and H == 128 and W == 128 and C == 128

    MW = _max_width(boxes)
    nc = _get_program(C, K, MW)

    s1_np = np.float16 if S1_DTYPE == F16 else np.float32
    in_maps = []
    per_image = []
    for n in range(N):
        WY, WXF, width, tall_idx = _build_image_weights(boxes[n], H, W, MW)
        per_image.append((width, tall_idx))
        img = np.ascontiguousarray(
            x[n].transpose(1, 0, 2).reshape(H, C * W))  # [h, (c, w)]
        WXFP = np.zeros((W, K * 128), np.float16)
        for k in range(K):
            WXFP[:, k * 128:k * 128 + 2 * MW] = WXF[:, k * 2 * MW:(k + 1) * 2 * MW]
        in_maps.append({
            "img": img.astype(s1_np),
            "wy": WY.astype(s1_np),
            "wxf": WXFP,
        })

    res = bass_utils.run_bass_kernel_spmd(
        nc, in_maps, core_ids=list(range(N_CORES)), **(_run_kwargs or {}))

    feats = np.empty((N, K, 2, C, PH, MW), np.float32)
    widths = np.empty((N, K, 2), np.float32)
    CH = C // 2
    for n in range(N):
        # device scratch layout: [k, jd, (h, c', i)] with c = h*CH + c';
        # jd = d*MW + j (rows 2*MW..127 are padding); dir1 row is PH-1-i
        s = res.results[n]["feats"].reshape(K, 128, 2, CH, PH)
        s = s[:, :2 * MW].reshape(K, 2, MW, 2, CH, PH)
        t = s.transpose(0, 1, 3, 4, 5, 2)  # (k, d, h, c', i, j)
        t = np.concatenate([t[:, :1], t[:, 1:, :, :, ::-1, :]], axis=1)
        feats[n] = t.reshape(K, 2, C, PH, MW).astype(np.float32)
        width, tall_idx = per_image[n]
        for k in tall_idx:
            feats[n, k] = _tall_feats(x[n], boxes[n], k, H, W, MW)
        widths[n] = width.astype(np.float32)[:, None]
    kernel.last_result = res
    return feats, widths
